# revision 72
# baseline (speedup 1.0000x reference)
"""Trainium2 Bass kernel for nn_ChemistryAwareDecoder.

Reference computation (per edge e = (s, d)):
    sp = z[s] * z[d]                       # [128]
    cp = chem[s] * chem[d]                 # [768]
    score_s = relu(sp @ sw1 + sb1) @ sw2 + sb2
    score_c = relu(cp @ cw1 + cb1) @ cw2 + cb2
    score_m = relu(concat(sp, cp) @ mw1 + mb1) @ mw2 + mb2
    t = w0*score_s + w1*score_c + w2*score_m
    bv = mask[s] * mask[d]
    out = bv > 0.5 ? t : score_s

Strategy: data-parallel over edges across 8 NeuronCores, with edges SPLIT BY
MASK CLASS on the host:
  - "full" edges (mask[s]&mask[d], ~25%): need the blended t-score -> gather
    1024B rows (z bf16 256B + chem fp8 768B) for both endpoints and run all
    three MLP paths (chem layers as fp8 DoubleRow matmuls).
  - "z-only" edges (~75%): output is score_s alone -> gather only 256B z rows
    and run just the structural path.  This cuts gather bytes ~2.3x and
    compute ~3x vs. scoring every edge fully (the transposing-gather drain
    rate is ~15-17 GB/s/engine regardless of row size, so time ~ bytes).

Within a core, each class is sorted by dst so each gather's dst indices fit a
32768-row int16 window of the node table; src always fits the core's 32768-row
shard window (edges are sharded by sorted src).  Transposing dma_gathers land
rows feature-major: [128 partitions, chunks, idx].  Chem products are written
contiguously interleaved (feature pair per u16) and fed to DoubleRow matmuls
as [p, j(stride 1), i(stride 2)] which streams at full PE rate.  Scores are
copied PSUM->SBUF one block deferred; the host adds scalar biases and
unpermutes (no mask blend needed on device - classes are pre-split).

Pacing (measured): the serial Pool-engine gather issue chain (~3us per
gather instruction: Q7 descriptor generation at ~10ns/desc/lane + ~1us
fixed) is the wall, so blocks are grouped 7-blocks/4-gathers of 896 idx
(56 descs per 16KB..56KB engine-packet; 64 descs or >=64KB packets HANG
the SDMA engines - that is also why single_packet=False and 4 SWDGE
queues, which corrupt gathers nondeterministically, are avoided).
HW exec ~259us vs 300us for the previous all-edges-full kernel.
"""

import os
import numpy as np

N_NODES = 100000
E_TOTAL = 200000
SD = 128
CD = 768
ROW_U16 = 512          # full row: 1024B = 128 bf16 z + 768 fp8 chem
ZROW_U16 = 128         # z-only row: 256B = 128 bf16 z
NCORES = 8
BLKF = 512             # full-class block (edges)
BLKZ = 512             # z-class block (edges)
SRCWIN = 32768
DSTWIN = 32768
WPACK_B = 2340

LAST_EXEC_NS = None


def _groups(nblk):
    """Split nblk 512-edge blocks into gather groups: (n_blocks, n_gathers,
    gather_width).  896 idx = 56 descs/engine-packet is the largest safe
    gather (64 descs hangs the SDMA; 48 was the old conservative max)."""
    out = []
    b = 0
    while nblk - b >= 7:
        out.append((7, 4, 896))
        b += 7
    while nblk - b >= 3:
        out.append((3, 2, 768))
        b += 3
    while nblk - b >= 1:
        out.append((1, 1, 512))
        b += 1
    return out


def _schedule(nbf, nbz):
    """Gather schedule entries: (kind, width_idx, eidx_col, edge_lo).
    eidx layout per gather: width/16 cols of src idx then width/16 of dst.
    """
    sched = []
    col = 0
    for kind, nblk in (("f", nbf), ("z", nbz)):
        elo = 0
        for gb, gg, w in _groups(nblk):
            for g in range(gg):
                sched.append((kind, w, col, elo))
                col += w // 8
                elo += w
    return sched, col


def _quantile_bases(sched, nbf, nbz, n_nodes):
    """Input-independent dst-window bases: each class is dst-sorted per core,
    so gather k's dst values cluster at its quantile of [0, n_nodes)."""
    bases = []
    for kind, w, col, elo in sched:
        tot = (nbf if kind == "f" else nbz) * BLKF
        mid = int(round((elo + w / 2) / tot * n_nodes))
        bases.append(max(0, min(mid - DSTWIN // 2, n_nodes - DSTWIN)))
    return tuple(bases)


def _build(n_nodes, nbf, nbz, bases, srcwin):
    import concourse.bass as bass  # noqa: F401
    import concourse.tile as tile
    from concourse import bacc, mybir
    from concourse.tile_rust import add_dep_helper

    F32 = mybir.dt.float32
    I16 = mybir.dt.int16
    U16 = mybir.dt.uint16
    U8 = mybir.dt.uint8
    BF16 = mybir.dt.bfloat16
    FP8 = mybir.dt.float8e4
    AF = mybir.ActivationFunctionType
    OP = mybir.AluOpType
    DR = mybir.MatmulPerfMode.DoubleRow

    sched, ecols = _schedule(nbf, nbz)
    nc = bacc.Bacc(num_swdge_queues=2, dynamic_dma_scratch_size=32768)

    table_d = nc.declare_dram_parameter("table", [n_nodes, ROW_U16], U16, isOutput=False)
    stable_d = nc.declare_dram_parameter("stable", [srcwin, ROW_U16], U16, isOutput=False)
    ztab_d = nc.declare_dram_parameter("ztab", [n_nodes, ZROW_U16], U16, isOutput=False)
    zstable_d = nc.declare_dram_parameter("zstable", [srcwin, ZROW_U16], U16, isOutput=False)
    eidx_d = nc.declare_dram_parameter("eidx", [128, ecols], I16, isOutput=False)
    wpack_d = nc.declare_dram_parameter("wpack", [128, WPACK_B], U8, isOutput=False)
    out_d = nc.declare_dram_parameter("out", [nbf + nbz, BLKF], F32, isOutput=True)

    with tile.TileContext(nc) as tc:
        with (
            tc.tile_pool(name="const", bufs=1) as cpool,
            tc.tile_pool(name="fgather", bufs=5) as fgpool,
            tc.tile_pool(name="zgather", bufs=12) as zgpool,
            tc.tile_pool(name="prod", bufs=6) as ppool,
            tc.tile_pool(name="hid", bufs=3) as hpool,
            tc.tile_pool(name="outp", bufs=3) as opool,
            tc.tile_pool(name="ph", bufs=2, space="PSUM") as phpool,
            tc.tile_pool(name="ps", bufs=2, space="PSUM") as pspool,
        ):
            # ---- constants: first gather's eidx chunk first, then the rest,
            # then the packed weights (single DMAs each).
            c0w = sched[0][1] // 8
            eidx_0 = cpool.tile([128, c0w], I16, tag="eidx0")
            nc.sync.dma_start(out=eidx_0[:], in_=eidx_d[:, 0:c0w])
            eidx_rest = cpool.tile([128, ecols - c0w], I16, tag="eidxrest")
            nc.sync.dma_start(out=eidx_rest[:], in_=eidx_d[:, c0w:])

            def eidx_at(col, w):
                if col < c0w:
                    return eidx_0[:, col:col + w]
                return eidx_rest[:, col - c0w:col - c0w + w]

            wpk_t = cpool.tile([128, WPACK_B], U8, tag="wpack")
            nc.sync.dma_start(out=wpk_t[:], in_=wpack_d[:])
            sw1_t = wpk_t[:, 0:128].bitcast(BF16)
            mw1z_t = wpk_t[:, 128:384].bitcast(BF16)
            chaw_f = wpk_t[:, 384:1152].bitcast(FP8).rearrange(
                "p (cj m) -> p cj m", cj=6)
            chbw_f = wpk_t[:, 1152:1536].bitcast(FP8).rearrange(
                "p (cj m) -> p cj m", cj=6)
            cbcw_f = wpk_t[:, 1536:2304].bitcast(FP8).rearrange(
                "p (cj m) -> p cj m", cj=6)
            w2f_t = wpk_t[:, 2304:2312].bitcast(BF16)[:, 0:3]
            biasA_t = wpk_t[:, 2320:2324].bitcast(F32)
            biasB_t = wpk_t[:, 2324:2328].bitcast(F32)
            biasC_t = wpk_t[:, 2328:2332].bitcast(F32)
            biasZ_t = wpk_t[0:64, 2332:2336].bitcast(F32)
            w2z_t = wpk_t[0:64, 2336:2338].bitcast(BF16)

            def emit_out(st):
                pscore, row, n, on_dve = st
                o_t = opool.tile([1, BLKF], F32, tag="o")
                nc.scalar.copy(out=o_t[:, 0:n], in_=pscore[0:1, 0:n])
                nc.sync.dma_start(out=out_d[row:row + 1, :], in_=o_t[:])

            def _fblock(b, segs, pending):
                # z product (bf16): chunk 0 of each gather segment
                prod_z = ppool.tile([128, BLKF], BF16, tag="prodz")
                off = 0
                for (sT, dT, lo, hi) in segs:
                    n = hi - lo
                    nc.vector.tensor_tensor(
                        out=prod_z[:, off:off + n],
                        in0=sT[:, lo:hi].bitcast(BF16),
                        in1=dT[:, lo:hi].bitcast(BF16),
                        op=OP.mult)
                    off += n

                # chem product (fp8), contiguous interleaved: byte (c, 2i+j)
                # = product of chem feature 256c+2a+j of edge i (partition a).
                prod_c = ppool.tile([128, 6 * BLKF], FP8, tag="prodc")
                out_c3 = prod_c[:].rearrange("p (c b) -> p c b", c=3)
                off = 0
                for (sT, dT, lo, hi) in segs:
                    n = hi - lo

                    def chem_view(t):
                        return t[:].rearrange("p (c i) -> p c i", c=4)[
                            :, 1:4, lo:hi].bitcast(FP8)
                    nc.vector.tensor_tensor(
                        out=out_c3[:, :, 2 * off:2 * (off + n)],
                        in0=chem_view(sT), in1=chem_view(dT),
                        op=OP.mult)
                    off += n
                prod_cv = prod_c[:].rearrange("p (c i j) -> p c j i",
                                              c=3, i=BLKF, j=2)

                def rhs_c(c):
                    return prod_cv[:, c]

                if pending is not None:
                    emit_out(pending)

                # first layers; pstb holds chb rows 0:64 (DoubleRow must hit
                # partition 0) and st rows 64:128
                pstb = phpool.tile([128, BLKF], F32, tag="pstb")
                i_st = nc.tensor.matmul(pstb[64:128, :], lhsT=sw1_t,
                                        rhs=prod_z[:], start=True, stop=True)
                for c in range(3):
                    i_mm = nc.tensor.matmul(
                        pstb[0:64, :], lhsT=chbw_f[:, 2 * c:2 * c + 2, :],
                        rhs=rhs_c(c),
                        start=(c == 0), stop=(c == 2), perf_mode=DR)
                    if c == 0:
                        add_dep_helper(i_mm.ins, i_st.ins, sync=False,
                                       reason="st bank-clear before chb accum")
                p_cha = phpool.tile([128, BLKF], F32, tag="pcha")
                for c in range(3):
                    nc.tensor.matmul(
                        p_cha[:], lhsT=chaw_f[:, 2 * c:2 * c + 2, :],
                        rhs=rhs_c(c),
                        start=(c == 0), stop=(c == 2), perf_mode=DR)
                p_cb = phpool.tile([128, BLKF], F32, tag="pcb")
                nc.tensor.matmul(p_cb[:], lhsT=mw1z_t, rhs=prod_z[:],
                                 start=True, stop=False)
                for c in range(3):
                    nc.tensor.matmul(
                        p_cb[:], lhsT=cbcw_f[:, 2 * c:2 * c + 2, :],
                        rhs=rhs_c(c),
                        start=False, stop=(c == 2), perf_mode=DR)

                hidA = hpool.tile([128, BLKF], BF16, tag="hidA")
                nc.scalar.activation(out=hidA[:], in_=pstb[:],
                                     func=AF.Relu, bias=biasA_t)
                hidB = hpool.tile([128, BLKF], BF16, tag="hidB")
                nc.scalar.activation(out=hidB[:], in_=p_cha[:],
                                     func=AF.Relu, bias=biasB_t)
                hidC = hpool.tile([128, BLKF], BF16, tag="hidC")
                nc.scalar.activation(out=hidC[:], in_=p_cb[:],
                                     func=AF.Relu, bias=biasC_t)

                # second layer: single t row (blend numerator)
                pscore = pspool.tile([128, BLKF], F32, tag="pscore")
                nc.tensor.matmul(pscore[0:1, :], lhsT=w2f_t[:, 0:1],
                                 rhs=hidA[:], start=True, stop=False)
                nc.tensor.matmul(pscore[0:1, :], lhsT=w2f_t[:, 1:2],
                                 rhs=hidB[:], start=False, stop=False)
                nc.tensor.matmul(pscore[0:1, :], lhsT=w2f_t[:, 2:3],
                                 rhs=hidC[:], start=False, stop=True)

                return (pscore, b, BLKF, False)

            def _zblock(zb, segs, pending):
                prod_z = ppool.tile([128, BLKZ], BF16, tag="prodzz")
                off = 0
                for (sT, dT, lo, hi) in segs:
                    n = hi - lo
                    nc.vector.tensor_tensor(
                        out=prod_z[:, off:off + n],
                        in0=sT[:, lo:hi].bitcast(BF16),
                        in1=dT[:, lo:hi].bitcast(BF16),
                        op=OP.mult)
                    off += n

                if pending is not None:
                    emit_out(pending)

                pz = phpool.tile([128, BLKZ], F32, tag="pcha")
                nc.tensor.matmul(pz[0:64, :], lhsT=sw1_t, rhs=prod_z[:],
                                 start=True, stop=True)
                hidZ = hpool.tile([64, BLKZ], BF16, tag="hidZ")
                nc.scalar.activation(out=hidZ[:], in_=pz[0:64, :],
                                     func=AF.Relu, bias=biasZ_t)
                psz = pspool.tile([128, BLKZ], F32, tag="pscore")
                nc.tensor.matmul(psz[0:1, :], lhsT=w2z_t, rhs=hidZ[:],
                                 start=True, stop=True)
                return (psz, nbf + zb, BLKZ, True)

            widx_regs = {}

            def emit_gathers(kind, sT, dT, w, col, base):
                if w not in widx_regs:
                    widx_regs[w] = nc.gpsimd.to_reg(w)
                wreg = widx_regs[w]
                # transposing gathers: out[a, c, i] = row_{idx_i} u16[c*128+a]
                # both classes on queues 0/1: splitting classes onto queues
                # 2/3 ran ~3% faster but corrupted gathers nondeterministically
                if kind == "f":
                    s_tab, d_tab, es = stable_d, table_d, ROW_U16
                else:
                    s_tab, d_tab, es = zstable_d, ztab_d, ZROW_U16
                q0, q1 = 0, 1
                nc.gpsimd.dma_gather(
                    out_ap=sT[:].rearrange("p (c e) -> p c e", e=w),
                    in_ap=s_tab[:],
                    idxs_ap=eidx_at(col, w // 16),
                    num_idxs=w, num_idxs_reg=wreg,
                    elem_size=es, transpose=True,
                    queue_num=q0,
                )
                nc.gpsimd.dma_gather(
                    out_ap=dT[:].rearrange("p (c e) -> p c e", e=w),
                    in_ap=d_tab[base:base + DSTWIN, :],
                    idxs_ap=eidx_at(col + w // 16, w // 16),
                    num_idxs=w, num_idxs_reg=wreg,
                    elem_size=es, transpose=True,
                    queue_num=q1,
                )

            # ---- pipeline: groups of blocks fed by gather pairs (7 blocks /
            # 4x896, 3 / 2x768, 1 / 1x512).  The f and z classes are
            # INTERLEAVED so z desc-gen (Pool-bound) fills the bubbles while
            # the PE chews on f blocks, and vice versa.
            fgroups = _groups(nbf)
            zgroups = _groups(nbz)
            # NOTE: interleaving f/z groups here trips Tile's per-queue DMA
            # semaphore lane locking (sim rejects; likely the 4-queue
            # corruption mechanism) — keep classes sequential.  The overlap
            # comes from deep gather pools instead: all f desc-gen runs
            # ahead, so the Pool engine starts z desc-gen while the PE is
            # still chewing on f blocks.
            merged = ([("f", i, g) for i, g in enumerate(fgroups)] +
                      [("z", i, g) for i, g in enumerate(zgroups)])
            # per-class sched entry offsets (sched lists f entries then z)
            nfe = sum(gg for _, gg, _ in fgroups)
            si_of = {"f": 0, "z": nfe}
            b0_of = {"f": 0, "z": 0}
            pending = None
            for kind, _, (gb, gg, w) in merged:
                blockfn = _fblock if kind == "f" else _zblock
                chunks = 4 if kind == "f" else 1
                si = si_of[kind]
                b0 = b0_of[kind]
                gpool = fgpool if kind == "f" else zgpool
                tiles = []
                for g in range(gg):
                    k, ww, col, elo = sched[si]
                    assert k == kind and ww == w
                    sT = gpool.tile([128, chunks * w], U16, tag=f"{kind}src")
                    dT = gpool.tile([128, chunks * w], U16, tag=f"{kind}dst")
                    emit_gathers(kind, sT, dT, w, col, bases[si])
                    tiles.append((sT, dT))
                    si += 1
                for j in range(gb):
                    elo, ehi = j * BLKF, (j + 1) * BLKF
                    segs = []
                    for g in range(gg):
                        glo, ghi = g * w, (g + 1) * w
                        a, b = max(elo, glo), min(ehi, ghi)
                        if a < b:
                            segs.append((tiles[g][0], tiles[g][1],
                                         a - glo, b - glo))
                    pending = blockfn(b0 + j, segs, pending)
                si_of[kind] = si
                b0_of[kind] = b0 + gb
            emit_out(pending)

    nc.finalize()
    return nc


def _host_prep(z, chemistry, edge, smiles_mask,
               sw1, sb1, sw2, sb2, cw1, cb1, cw2, cb2, mw1, mb1, mw2, mb2,
               path_weights, n_nodes=N_NODES, ncores=NCORES):
    """Sort/split edges, build packed tables + per-core shards."""
    import ml_dtypes
    bf16 = ml_dtypes.bfloat16
    e4m3 = ml_dtypes.float8_e4m3

    z = np.asarray(z, np.float32)
    chemistry = np.asarray(chemistry, np.float32)
    mask = np.asarray(smiles_mask, np.int64).reshape(-1)

    table = np.empty((n_nodes, ROW_U16), '<u2')
    tb8 = table.view(np.uint8)
    zbytes = np.ascontiguousarray(z.astype(bf16)).view(np.uint8)
    tb8[:, :2 * SD] = zbytes
    tb8[:, 2 * SD:] = np.ascontiguousarray(chemistry.astype(e4m3)).view(np.uint8)
    ztab = np.ascontiguousarray(zbytes).view('<u2')
    assert ztab.shape == (n_nodes, ZROW_U16)

    srcwin = min(SRCWIN, n_nodes)

    pw = np.asarray(path_weights, np.float64)
    e = np.exp(pw - pw.max())
    w = e / e.sum()
    w0, w1, w2 = [float(x) for x in w]

    sw1 = np.asarray(sw1, np.float64)
    cw1 = np.asarray(cw1, np.float64)
    mw1 = np.asarray(mw1, np.float64)

    def dr_pack(W):
        # W: [768, M] -> [128, 6, M] with [a, 2c+j, m] = W[256c+2a+j, m]
        M = W.shape[1]
        out = np.empty((128, 6, M), np.float64)
        for c in range(3):
            blkw = W[256 * c:256 * (c + 1)]          # [256, M]
            out[:, 2 * c + 0] = blkw[0::2]
            out[:, 2 * c + 1] = blkw[1::2]
        return np.ascontiguousarray(out.astype(e4m3)).view(np.uint8).reshape(128, 6 * M)

    chaw = dr_pack(cw1[:, :128])
    chbw = dr_pack(cw1[:, 128:192])
    cbcw = dr_pack(mw1[128:, :])

    biasA = np.concatenate([np.asarray(cb1, np.float32)[128:],
                            np.asarray(sb1, np.float32)]).astype(np.float32)
    biasB = np.asarray(cb1, np.float32)[:128].astype(np.float32)
    biasC = np.asarray(mb1, np.float32).astype(np.float32)
    biasZ = np.zeros(128, np.float32)
    biasZ[0:64] = np.asarray(sb1, np.float32)
    assert biasA.shape == biasB.shape == biasC.shape == (128,)

    sw2v = np.asarray(sw2, np.float64).reshape(-1)
    cw2v = np.asarray(cw2, np.float64).reshape(-1)
    mw2v = np.asarray(mw2, np.float64).reshape(-1)
    sb2v = float(np.asarray(sb2, np.float64).reshape(())[()])
    cb2v = float(np.asarray(cb2, np.float64).reshape(())[()])
    mb2v = float(np.asarray(mb2, np.float64).reshape(())[()])
    tbias = w0 * sb2v + w1 * cb2v + w2 * mb2v

    # w2f: 3 cols for the t-score; col0 pairs with hidA=[chb|st]
    w2f = np.zeros((128, 3), np.float64)
    w2f[0:64, 0] = w1 * cw2v[128:]
    w2f[64:128, 0] = w0 * sw2v
    w2f[:, 1] = w1 * cw2v[:128]
    w2f[:, 2] = w2 * mw2v
    w2f = w2f.astype(bf16)
    w2z = np.zeros((128, 1), np.float64)
    w2z[0:64, 0] = sw2v
    w2z = w2z.astype(bf16)

    wpack = np.zeros((128, WPACK_B), np.uint8)
    wpack[:, 0:128] = np.ascontiguousarray(sw1.astype(bf16)).view(np.uint8)
    wpack[:, 128:384] = np.ascontiguousarray(mw1[:128].astype(bf16)).view(np.uint8)
    wpack[:, 384:1152] = chaw
    wpack[:, 1152:1536] = chbw
    wpack[:, 1536:2304] = cbcw
    wpack[:, 2304:2310] = np.ascontiguousarray(w2f).view(np.uint8)
    wpack[:, 2320:2324] = biasA.reshape(128, 1).view(np.uint8)
    wpack[:, 2324:2328] = biasB.reshape(128, 1).view(np.uint8)
    wpack[:, 2328:2332] = biasC.reshape(128, 1).view(np.uint8)
    wpack[:, 2332:2336] = biasZ.reshape(128, 1).view(np.uint8)
    wpack[:, 2336:2338] = np.ascontiguousarray(w2z).view(np.uint8)

    edge = np.asarray(edge)
    E = edge.shape[0]
    src_all = edge[:, 0].astype(np.int64)
    dst_all = edge[:, 1].astype(np.int64)
    order = np.argsort(src_all, kind='stable')
    epc = E // ncores

    # per-core class split (both-valid -> full, else z-only)
    cores = []
    force_full = os.environ.get("KERNEL_FORCE_FULL", "0") == "1"
    for c in range(ncores):
        ids = order[c * epc:(c + 1) * epc]
        bv = (mask[src_all[ids]] * mask[dst_all[ids]]) > 0
        if force_full:
            bv = np.ones_like(bv)
        cores.append((ids[bv], ids[~bv]))
    nbf = max(1, max((len(f) + BLKF - 1) // BLKF for f, _ in cores))
    nbz = max(0 if force_full else 1,
              max((len(zc) + BLKZ - 1) // BLKZ for _, zc in cores))

    sched, ecols = _schedule(nbf, nbz)
    bases = _quantile_bases(sched, nbf, nbz, n_nodes)

    def pack_idx(vals):
        # wrap into 16 partitions: eidx[i%16, col + i//16] = vals[i]
        n = len(vals)
        out = np.zeros((16, n // 16), np.int16)
        ar = np.arange(n)
        out[ar % 16, ar // 16] = vals
        return out

    shards = []
    for c in range(ncores):
        fid, zid = cores[c]
        allids = np.concatenate([fid, zid])
        s_all = src_all[allids]
        w0c = max(0, min(int(s_all.min()), n_nodes - srcwin))
        assert int(s_all.max()) - w0c < srcwin, "src window overflow"

        def classprep(ids, nblk, blk, gblks):
            npad = nblk * blk
            s = src_all[ids]
            d = np.asarray(dst_all[ids])
            dord = np.argsort(d, kind='stable')
            ids, s, d = ids[dord], s[dord], d[dord]
            sp = np.zeros(npad, np.int64)
            dp = np.zeros(npad, np.int64)
            perm = np.full(npad, -1, np.int64)
            sp[:len(ids)] = s - w0c
            dp[:len(ids)] = d
            perm[:len(ids)] = ids
            if len(ids) < npad:
                sp[len(ids):] = s[-1] - w0c if len(s) else 0
                # empty class: put pad dsts at each gather's window center
                dp[len(ids):] = d[-1] if len(d) else 0
                if not len(d):
                    ar = np.arange(npad)
                    g0 = (ar // (gblks * blk)) * (gblks * blk)
                    wg = np.minimum(gblks * blk, npad - g0)
                    dp[:] = np.minimum((g0 + wg // 2) * n_nodes // npad,
                                       n_nodes - 1)
            return sp, dp, perm

        fsp, fdp, fperm = classprep(fid, nbf, BLKF, 1)
        zsp, zdp, zperm = classprep(zid, nbz, BLKZ, 1)

        eidx = np.zeros((16, ecols), np.int16)
        for (kind, w, col, elo), base in zip(sched, bases):
            if kind == "f":
                sp = fsp[elo: elo + w]
                dp = fdp[elo: elo + w]
            else:
                sp = zsp[elo: elo + w]
                dp = zdp[elo: elo + w]
            assert dp.min() - base >= 0 and dp.max() - base < DSTWIN, \
                "dst window overflow"
            assert sp.min() >= 0 and sp.max() < srcwin
            eidx[:, col:col + w // 16] = pack_idx(sp)
            eidx[:, col + w // 16:col + w // 8] = pack_idx(dp - base)
        eidx = np.tile(eidx, (8, 1))
        stable = np.ascontiguousarray(table[w0c:w0c + srcwin])
        zstable = np.ascontiguousarray(ztab[w0c:w0c + srcwin])
        shards.append((eidx, stable, zstable, fperm, zperm))

    shared = dict(table=table, ztab=ztab, wpack=wpack)
    meta = dict(nbf=nbf, nbz=nbz, srcwin=srcwin, sched=sched,
                bases=bases, sb2v=sb2v, tbias=tbias, E=E)
    return shared, shards, meta


_BUILD_CACHE = {}


def kernel(z, chemistry, edge, smiles_mask,
           sw1, sb1, sw2, sb2, cw1, cb1, cw2, cb2, mw1, mb1, mw2, mb2,
           path_weights):
    global LAST_EXEC_NS
    from concourse import bass_utils
    from concourse.bass_utils import run_bass_kernel_spmd

    trace = os.environ.get("KERNEL_TRACE", "0") == "1"
    if trace:
        bass_utils.upload_artifacts = lambda tmpdir: tmpdir

    shared, shards, meta = _host_prep(
        z, chemistry, edge, smiles_mask, sw1, sb1, sw2, sb2,
        cw1, cb1, cw2, cb2, mw1, mb1, mw2, mb2, path_weights)
    nbf, nbz, srcwin = meta["nbf"], meta["nbz"], meta["srcwin"]

    key = (N_NODES, nbf, nbz, meta["bases"], srcwin)
    if key not in _BUILD_CACHE:
        _BUILD_CACHE[key] = _build(N_NODES, nbf, nbz, meta["bases"], srcwin)
    nc = _BUILD_CACHE[key]
    in_maps = []
    for c in range(NCORES):
        m = dict(shared)
        m["eidx"], m["stable"], m["zstable"] = \
            shards[c][0], shards[c][1], shards[c][2]
        in_maps.append(m)
    tmpdir = os.environ.get("KERNEL_TRACE_DIR") or None
    res = run_bass_kernel_spmd(nc, in_maps, core_ids=list(range(NCORES)),
                               trace=trace, tmpdir=tmpdir)
    if trace:
        LAST_EXEC_NS = res.exec_time_ns
    results = [res.results[c]["out"].astype(np.float32) for c in range(NCORES)]

    E = meta["E"]
    sb2v, tbias = meta["sb2v"], meta["tbias"]
    result = np.zeros(E, np.float32)
    for c in range(NCORES):
        fperm, zperm = shards[c][3], shards[c][4]
        dev = results[c].reshape(-1)
        t_sc = dev[0:nbf * BLKF] + np.float32(tbias)
        s_sc = dev[nbf * BLKF:] + np.float32(sb2v)
        fv = fperm >= 0
        result[fperm[fv]] = t_sc[fv]
        zv = zperm >= 0
        result[zperm[zv]] = s_sc[zv]
    return result


# revision 74
# speedup vs baseline: 1.0184x; 1.0184x over previous
"""Trainium2 Bass kernel for nn_ChemistryAwareDecoder.

Reference computation (per edge e = (s, d)):
    sp = z[s] * z[d]                       # [128]
    cp = chem[s] * chem[d]                 # [768]
    score_s = relu(sp @ sw1 + sb1) @ sw2 + sb2
    score_c = relu(cp @ cw1 + cb1) @ cw2 + cb2
    score_m = relu(concat(sp, cp) @ mw1 + mb1) @ mw2 + mb2
    t = w0*score_s + w1*score_c + w2*score_m
    bv = mask[s] * mask[d]
    out = bv > 0.5 ? t : score_s

Strategy: data-parallel over edges across 8 NeuronCores, with edges SPLIT BY
MASK CLASS on the host:
  - "full" edges (mask[s]&mask[d], ~25%): need the blended t-score -> gather
    1024B rows (z bf16 256B + chem fp8 768B) for both endpoints and run all
    three MLP paths (chem layers as fp8 DoubleRow matmuls).
  - "z-only" edges (~75%): output is score_s alone -> gather only 256B z rows
    and run just the structural path.  This cuts gather bytes ~2.3x and
    compute ~3x vs. scoring every edge fully (the transposing-gather drain
    rate is ~15-17 GB/s/engine regardless of row size, so time ~ bytes).

Within a core, each class is sorted by dst so each gather's dst indices fit a
32768-row int16 window of the node table; src always fits the core's 32768-row
shard window (edges are sharded by sorted src).  Transposing dma_gathers land
rows feature-major: [128 partitions, chunks, idx].  Chem products are written
contiguously interleaved (feature pair per u16) and fed to DoubleRow matmuls
as [p, j(stride 1), i(stride 2)] which streams at full PE rate.  Scores are
copied PSUM->SBUF one block deferred; the host adds scalar biases and
unpermutes (no mask blend needed on device - classes are pre-split).

Pacing (measured): the serial Pool-engine gather issue chain (~3us per
gather instruction: Q7 descriptor generation at ~10ns/desc/lane + ~1us
fixed) is the wall, so blocks are grouped 7-blocks/4-gathers of 896 idx
(56 descs per 16KB..56KB engine-packet; 64 descs or >=64KB packets HANG
the SDMA engines - that is also why single_packet=False and 4 SWDGE
queues, which corrupt gathers nondeterministically, are avoided).
HW exec ~259us vs 300us for the previous all-edges-full kernel.
"""

import os
import numpy as np

N_NODES = 100000
E_TOTAL = 200000
SD = 128
CD = 768
ROW_U16 = 512          # full row: 1024B = 128 bf16 z + 768 fp8 chem
ZROW_U16 = 128         # z-only row: 256B = 128 bf16 z
NCORES = 8
BLKF = 512             # full-class block (edges)
BLKZ = 512             # z-class block (edges)
SRCWIN = 32768
DSTWIN = 32768
WPACK_B = 2340

LAST_EXEC_NS = None


def _groups(nblk):
    """Split nblk 512-edge blocks into gather groups: (n_blocks, n_gathers,
    gather_width).  896 idx = 56 descs/engine-packet is the largest safe
    gather (64 descs hangs the SDMA; 48 was the old conservative max)."""
    out = []
    b = 0
    while nblk - b >= 7:
        out.append((7, 4, 896))
        b += 7
    while nblk - b >= 3:
        out.append((3, 2, 768))
        b += 3
    while nblk - b >= 1:
        out.append((1, 1, 512))
        b += 1
    return out


def _schedule(nbf, nbz):
    """Gather schedule entries: (kind, width_idx, eidx_col, edge_lo).
    eidx layout per gather: width/16 cols of src idx then width/16 of dst.
    """
    sched = []
    col = 0
    for kind, nblk in (("f", nbf), ("z", nbz)):
        elo = 0
        for gb, gg, w in _groups(nblk):
            for g in range(gg):
                sched.append((kind, w, col, elo))
                col += w // 8
                elo += w
    return sched, col


def _quantile_bases(sched, nbf, nbz, n_nodes):
    """Input-independent dst-window bases: each class is dst-sorted per core,
    so gather k's dst values cluster at its quantile of [0, n_nodes)."""
    bases = []
    for kind, w, col, elo in sched:
        tot = (nbf if kind == "f" else nbz) * BLKF
        mid = int(round((elo + w / 2) / tot * n_nodes))
        bases.append(max(0, min(mid - DSTWIN // 2, n_nodes - DSTWIN)))
    return tuple(bases)


def _build(n_nodes, nbf, nbz, bases, srcwin):
    import concourse.bass as bass  # noqa: F401
    import concourse.tile as tile
    from concourse import bacc, mybir
    from concourse.tile_rust import add_dep_helper

    F32 = mybir.dt.float32
    I16 = mybir.dt.int16
    U16 = mybir.dt.uint16
    U8 = mybir.dt.uint8
    BF16 = mybir.dt.bfloat16
    FP8 = mybir.dt.float8e4
    AF = mybir.ActivationFunctionType
    OP = mybir.AluOpType
    DR = mybir.MatmulPerfMode.DoubleRow

    sched, ecols = _schedule(nbf, nbz)
    nc = bacc.Bacc(num_swdge_queues=2, dynamic_dma_scratch_size=32768)

    table_d = nc.declare_dram_parameter("table", [n_nodes, ROW_U16], U16, isOutput=False)
    stable_d = nc.declare_dram_parameter("stable", [srcwin, ROW_U16], U16, isOutput=False)
    ztab_d = nc.declare_dram_parameter("ztab", [n_nodes, ZROW_U16], U16, isOutput=False)
    zstable_d = nc.declare_dram_parameter("zstable", [srcwin, ZROW_U16], U16, isOutput=False)
    eidx_d = nc.declare_dram_parameter("eidx", [128, ecols], I16, isOutput=False)
    wpack_d = nc.declare_dram_parameter("wpack", [128, WPACK_B], U8, isOutput=False)
    out_d = nc.declare_dram_parameter("out", [nbf + nbz, BLKF], F32, isOutput=True)

    with tile.TileContext(nc) as tc:
        with (
            tc.tile_pool(name="const", bufs=1) as cpool,
            tc.tile_pool(name="fgather", bufs=5) as fgpool,
            tc.tile_pool(name="zgather", bufs=12) as zgpool,
            tc.tile_pool(name="prod", bufs=6) as ppool,
            tc.tile_pool(name="hid", bufs=3) as hpool,
            tc.tile_pool(name="outp", bufs=3) as opool,
            tc.tile_pool(name="ph", bufs=2, space="PSUM") as phpool,
            tc.tile_pool(name="ps", bufs=2, space="PSUM") as pspool,
        ):
            # ---- constants: first gather's eidx chunk first, then the rest,
            # then the packed weights (single DMAs each).
            c0w = sched[0][1] // 8
            eidx_0 = cpool.tile([128, c0w], I16, tag="eidx0")
            nc.sync.dma_start(out=eidx_0[:], in_=eidx_d[:, 0:c0w])
            eidx_rest = cpool.tile([128, ecols - c0w], I16, tag="eidxrest")
            nc.sync.dma_start(out=eidx_rest[:], in_=eidx_d[:, c0w:])

            def eidx_at(col, w):
                if col < c0w:
                    return eidx_0[:, col:col + w]
                return eidx_rest[:, col - c0w:col - c0w + w]

            wpk_t = cpool.tile([128, WPACK_B], U8, tag="wpack")
            nc.sync.dma_start(out=wpk_t[:], in_=wpack_d[:])
            sw1_t = wpk_t[:, 0:128].bitcast(BF16)
            mw1z_t = wpk_t[:, 128:384].bitcast(BF16)
            chaw_f = wpk_t[:, 384:1152].bitcast(FP8).rearrange(
                "p (cj m) -> p cj m", cj=6)
            chbw_f = wpk_t[:, 1152:1536].bitcast(FP8).rearrange(
                "p (cj m) -> p cj m", cj=6)
            cbcw_f = wpk_t[:, 1536:2304].bitcast(FP8).rearrange(
                "p (cj m) -> p cj m", cj=6)
            w2f_t = wpk_t[:, 2304:2312].bitcast(BF16)[:, 0:3]
            biasA_t = wpk_t[:, 2320:2324].bitcast(F32)
            biasB_t = wpk_t[:, 2324:2328].bitcast(F32)
            biasC_t = wpk_t[:, 2328:2332].bitcast(F32)
            biasZ_t = wpk_t[0:64, 2332:2336].bitcast(F32)
            w2z_t = wpk_t[0:64, 2336:2338].bitcast(BF16)

            def emit_out(st):
                pscore, row, n, on_dve = st
                o_t = opool.tile([1, BLKF], F32, tag="o")
                nc.scalar.copy(out=o_t[:, 0:n], in_=pscore[0:1, 0:n])
                nc.sync.dma_start(out=out_d[row:row + 1, :], in_=o_t[:])

            def _fblock(b, segs, pending):
                # z product (bf16): chunk 0 of each gather segment
                prod_z = ppool.tile([128, BLKF], BF16, tag="prodz")
                off = 0
                for (sT, dT, lo, hi) in segs:
                    n = hi - lo
                    nc.vector.tensor_tensor(
                        out=prod_z[:, off:off + n],
                        in0=sT[:, lo:hi].bitcast(BF16),
                        in1=dT[:, lo:hi].bitcast(BF16),
                        op=OP.mult)
                    off += n

                # chem product (fp8), contiguous interleaved: byte (c, 2i+j)
                # = product of chem feature 256c+2a+j of edge i (partition a).
                prod_c = ppool.tile([128, 6 * BLKF], FP8, tag="prodc")
                out_c3 = prod_c[:].rearrange("p (c b) -> p c b", c=3)
                off = 0
                for (sT, dT, lo, hi) in segs:
                    n = hi - lo

                    def chem_view(t):
                        return t[:].rearrange("p (c i) -> p c i", c=4)[
                            :, 1:4, lo:hi].bitcast(FP8)
                    nc.vector.tensor_tensor(
                        out=out_c3[:, :, 2 * off:2 * (off + n)],
                        in0=chem_view(sT), in1=chem_view(dT),
                        op=OP.mult)
                    off += n
                prod_cv = prod_c[:].rearrange("p (c i j) -> p c j i",
                                              c=3, i=BLKF, j=2)

                def rhs_c(c):
                    return prod_cv[:, c]

                if pending is not None:
                    emit_out(pending)

                # first layers; pstb holds chb rows 0:64 (DoubleRow must hit
                # partition 0) and st rows 64:128
                pstb = phpool.tile([128, BLKF], F32, tag="pstb")
                i_st = nc.tensor.matmul(pstb[64:128, :], lhsT=sw1_t,
                                        rhs=prod_z[:], start=True, stop=True)
                for c in range(3):
                    i_mm = nc.tensor.matmul(
                        pstb[0:64, :], lhsT=chbw_f[:, 2 * c:2 * c + 2, :],
                        rhs=rhs_c(c),
                        start=(c == 0), stop=(c == 2), perf_mode=DR)
                    if c == 0:
                        add_dep_helper(i_mm.ins, i_st.ins, sync=False,
                                       reason="st bank-clear before chb accum")
                p_cha = phpool.tile([128, BLKF], F32, tag="pcha")
                for c in range(3):
                    nc.tensor.matmul(
                        p_cha[:], lhsT=chaw_f[:, 2 * c:2 * c + 2, :],
                        rhs=rhs_c(c),
                        start=(c == 0), stop=(c == 2), perf_mode=DR)
                p_cb = phpool.tile([128, BLKF], F32, tag="pcb")
                nc.tensor.matmul(p_cb[:], lhsT=mw1z_t, rhs=prod_z[:],
                                 start=True, stop=False)
                for c in range(3):
                    nc.tensor.matmul(
                        p_cb[:], lhsT=cbcw_f[:, 2 * c:2 * c + 2, :],
                        rhs=rhs_c(c),
                        start=False, stop=(c == 2), perf_mode=DR)

                hidA = hpool.tile([128, BLKF], BF16, tag="hidA")
                nc.scalar.activation(out=hidA[:], in_=pstb[:],
                                     func=AF.Relu, bias=biasA_t)
                hidB = hpool.tile([128, BLKF], BF16, tag="hidB")
                nc.scalar.activation(out=hidB[:], in_=p_cha[:],
                                     func=AF.Relu, bias=biasB_t)
                hidC = hpool.tile([128, BLKF], BF16, tag="hidC")
                nc.scalar.activation(out=hidC[:], in_=p_cb[:],
                                     func=AF.Relu, bias=biasC_t)

                # second layer: single t row (blend numerator)
                pscore = pspool.tile([128, BLKF], F32, tag="pscore")
                nc.tensor.matmul(pscore[0:1, :], lhsT=w2f_t[:, 0:1],
                                 rhs=hidA[:], start=True, stop=False)
                nc.tensor.matmul(pscore[0:1, :], lhsT=w2f_t[:, 1:2],
                                 rhs=hidB[:], start=False, stop=False)
                nc.tensor.matmul(pscore[0:1, :], lhsT=w2f_t[:, 2:3],
                                 rhs=hidC[:], start=False, stop=True)

                return (pscore, b, BLKF, False)

            def _zblock(zb, segs, pending):
                prod_z = ppool.tile([128, BLKZ], BF16, tag="prodzz")
                off = 0
                for (sT, dT, lo, hi) in segs:
                    n = hi - lo
                    nc.vector.tensor_tensor(
                        out=prod_z[:, off:off + n],
                        in0=sT[:, lo:hi].bitcast(BF16),
                        in1=dT[:, lo:hi].bitcast(BF16),
                        op=OP.mult)
                    off += n

                if pending is not None:
                    emit_out(pending)

                pz = phpool.tile([128, BLKZ], F32, tag="pcha")
                nc.tensor.matmul(pz[0:64, :], lhsT=sw1_t, rhs=prod_z[:],
                                 start=True, stop=True)
                hidZ = hpool.tile([64, BLKZ], BF16, tag="hidZ")
                nc.scalar.activation(out=hidZ[:], in_=pz[0:64, :],
                                     func=AF.Relu, bias=biasZ_t)
                psz = pspool.tile([128, BLKZ], F32, tag="pscore")
                nc.tensor.matmul(psz[0:1, :], lhsT=w2z_t, rhs=hidZ[:],
                                 start=True, stop=True)
                return (psz, nbf + zb, BLKZ, True)

            widx_regs = {}

            def emit_gathers(kind, sT, dT, w, col, base):
                if w not in widx_regs:
                    widx_regs[w] = nc.gpsimd.to_reg(w)
                wreg = widx_regs[w]
                # transposing gathers: out[a, c, i] = row_{idx_i} u16[c*128+a]
                # both classes on queues 0/1: splitting classes onto queues
                # 2/3 ran ~3% faster but corrupted gathers nondeterministically
                if kind == "f":
                    s_tab, d_tab, es = stable_d, table_d, ROW_U16
                else:
                    s_tab, d_tab, es = zstable_d, ztab_d, ZROW_U16
                q0, q1 = 0, 1
                nc.gpsimd.dma_gather(
                    out_ap=sT[:].rearrange("p (c e) -> p c e", e=w),
                    in_ap=s_tab[:],
                    idxs_ap=eidx_at(col, w // 16),
                    num_idxs=w, num_idxs_reg=wreg,
                    elem_size=es, transpose=True,
                    queue_num=q0,
                )
                nc.gpsimd.dma_gather(
                    out_ap=dT[:].rearrange("p (c e) -> p c e", e=w),
                    in_ap=d_tab[base:base + DSTWIN, :],
                    idxs_ap=eidx_at(col + w // 16, w // 16),
                    num_idxs=w, num_idxs_reg=wreg,
                    elem_size=es, transpose=True,
                    queue_num=q1,
                )

            # ---- pipeline: groups of blocks fed by gather pairs (7 blocks /
            # 4x896, 3 / 2x768, 1 / 1x512).  The f and z classes are
            # INTERLEAVED so z desc-gen (Pool-bound) fills the bubbles while
            # the PE chews on f blocks, and vice versa.
            fgroups = _groups(nbf)
            zgroups = _groups(nbz)
            # NOTE: interleaving f/z groups here trips Tile's per-queue DMA
            # semaphore lane locking (sim rejects; likely the 4-queue
            # corruption mechanism) — keep classes sequential.  The overlap
            # comes from deep gather pools instead: all f desc-gen runs
            # ahead, so the Pool engine starts z desc-gen while the PE is
            # still chewing on f blocks.
            merged = ([("f", i, g) for i, g in enumerate(fgroups)] +
                      [("z", i, g) for i, g in enumerate(zgroups)])
            # per-class sched entry offsets (sched lists f entries then z)
            nfe = sum(gg for _, gg, _ in fgroups)
            si_of = {"f": 0, "z": nfe}
            b0_of = {"f": 0, "z": 0}
            pending = None
            for kind, _, (gb, gg, w) in merged:
                blockfn = _fblock if kind == "f" else _zblock
                chunks = 4 if kind == "f" else 1
                si = si_of[kind]
                b0 = b0_of[kind]
                gpool = fgpool if kind == "f" else zgpool
                tiles = []
                for g in range(gg):
                    k, ww, col, elo = sched[si]
                    assert k == kind and ww == w
                    sT = gpool.tile([128, chunks * w], U16, tag=f"{kind}src")
                    dT = gpool.tile([128, chunks * w], U16, tag=f"{kind}dst")
                    emit_gathers(kind, sT, dT, w, col, bases[si])
                    tiles.append((sT, dT))
                    si += 1
                for j in range(gb):
                    elo, ehi = j * BLKF, (j + 1) * BLKF
                    segs = []
                    for g in range(gg):
                        glo, ghi = g * w, (g + 1) * w
                        a, b = max(elo, glo), min(ehi, ghi)
                        if a < b:
                            segs.append((tiles[g][0], tiles[g][1],
                                         a - glo, b - glo))
                    pending = blockfn(b0 + j, segs, pending)
                si_of[kind] = si
                b0_of[kind] = b0 + gb
            emit_out(pending)

    nc.finalize()
    return nc


def _host_prep(z, chemistry, edge, smiles_mask,
               sw1, sb1, sw2, sb2, cw1, cb1, cw2, cb2, mw1, mb1, mw2, mb2,
               path_weights, n_nodes=N_NODES, ncores=NCORES):
    """Sort/split edges, build packed tables + per-core shards."""
    import ml_dtypes
    bf16 = ml_dtypes.bfloat16
    e4m3 = ml_dtypes.float8_e4m3

    z = np.asarray(z, np.float32)
    chemistry = np.asarray(chemistry, np.float32)
    mask = np.asarray(smiles_mask, np.int64).reshape(-1)

    table = np.empty((n_nodes, ROW_U16), '<u2')
    tb8 = table.view(np.uint8)
    zbytes = np.ascontiguousarray(z.astype(bf16)).view(np.uint8)
    tb8[:, :2 * SD] = zbytes
    tb8[:, 2 * SD:] = np.ascontiguousarray(chemistry.astype(e4m3)).view(np.uint8)
    ztab = np.ascontiguousarray(zbytes).view('<u2')
    assert ztab.shape == (n_nodes, ZROW_U16)

    srcwin = min(SRCWIN, n_nodes)

    pw = np.asarray(path_weights, np.float64)
    e = np.exp(pw - pw.max())
    w = e / e.sum()
    w0, w1, w2 = [float(x) for x in w]

    sw1 = np.asarray(sw1, np.float64)
    cw1 = np.asarray(cw1, np.float64)
    mw1 = np.asarray(mw1, np.float64)

    def dr_pack(W):
        # W: [768, M] -> [128, 6, M] with [a, 2c+j, m] = W[256c+2a+j, m]
        M = W.shape[1]
        out = np.empty((128, 6, M), np.float64)
        for c in range(3):
            blkw = W[256 * c:256 * (c + 1)]          # [256, M]
            out[:, 2 * c + 0] = blkw[0::2]
            out[:, 2 * c + 1] = blkw[1::2]
        return np.ascontiguousarray(out.astype(e4m3)).view(np.uint8).reshape(128, 6 * M)

    chaw = dr_pack(cw1[:, :128])
    chbw = dr_pack(cw1[:, 128:192])
    cbcw = dr_pack(mw1[128:, :])

    biasA = np.concatenate([np.asarray(cb1, np.float32)[128:],
                            np.asarray(sb1, np.float32)]).astype(np.float32)
    biasB = np.asarray(cb1, np.float32)[:128].astype(np.float32)
    biasC = np.asarray(mb1, np.float32).astype(np.float32)
    biasZ = np.zeros(128, np.float32)
    biasZ[0:64] = np.asarray(sb1, np.float32)
    assert biasA.shape == biasB.shape == biasC.shape == (128,)

    sw2v = np.asarray(sw2, np.float64).reshape(-1)
    cw2v = np.asarray(cw2, np.float64).reshape(-1)
    mw2v = np.asarray(mw2, np.float64).reshape(-1)
    sb2v = float(np.asarray(sb2, np.float64).reshape(())[()])
    cb2v = float(np.asarray(cb2, np.float64).reshape(())[()])
    mb2v = float(np.asarray(mb2, np.float64).reshape(())[()])
    tbias = w0 * sb2v + w1 * cb2v + w2 * mb2v

    # w2f: 3 cols for the t-score; col0 pairs with hidA=[chb|st]
    w2f = np.zeros((128, 3), np.float64)
    w2f[0:64, 0] = w1 * cw2v[128:]
    w2f[64:128, 0] = w0 * sw2v
    w2f[:, 1] = w1 * cw2v[:128]
    w2f[:, 2] = w2 * mw2v
    w2f = w2f.astype(bf16)
    w2z = np.zeros((128, 1), np.float64)
    w2z[0:64, 0] = sw2v
    w2z = w2z.astype(bf16)

    wpack = np.zeros((128, WPACK_B), np.uint8)
    wpack[:, 0:128] = np.ascontiguousarray(sw1.astype(bf16)).view(np.uint8)
    wpack[:, 128:384] = np.ascontiguousarray(mw1[:128].astype(bf16)).view(np.uint8)
    wpack[:, 384:1152] = chaw
    wpack[:, 1152:1536] = chbw
    wpack[:, 1536:2304] = cbcw
    wpack[:, 2304:2310] = np.ascontiguousarray(w2f).view(np.uint8)
    wpack[:, 2320:2324] = biasA.reshape(128, 1).view(np.uint8)
    wpack[:, 2324:2328] = biasB.reshape(128, 1).view(np.uint8)
    wpack[:, 2328:2332] = biasC.reshape(128, 1).view(np.uint8)
    wpack[:, 2332:2336] = biasZ.reshape(128, 1).view(np.uint8)
    wpack[:, 2336:2338] = np.ascontiguousarray(w2z).view(np.uint8)

    edge = np.asarray(edge)
    E = edge.shape[0]
    src_all = edge[:, 0].astype(np.int64)
    dst_all = edge[:, 1].astype(np.int64)
    order = np.argsort(src_all, kind='stable')
    epc = E // ncores

    # per-core class split (both-valid -> full, else z-only)
    cores = []
    force_full = os.environ.get("KERNEL_FORCE_FULL", "0") == "1"
    for c in range(ncores):
        ids = order[c * epc:(c + 1) * epc]
        bv = (mask[src_all[ids]] * mask[dst_all[ids]]) > 0
        if force_full:
            bv = np.ones_like(bv)
        cores.append((ids[bv], ids[~bv]))
    nbf = max(1, max((len(f) + BLKF - 1) // BLKF for f, _ in cores))
    nbz = max(0 if force_full else 1,
              max((len(zc) + BLKZ - 1) // BLKZ for _, zc in cores))

    sched, ecols = _schedule(nbf, nbz)
    bases = _quantile_bases(sched, nbf, nbz, n_nodes)

    def pack_idx(vals):
        # wrap into 16 partitions: eidx[i%16, col + i//16] = vals[i]
        n = len(vals)
        out = np.zeros((16, n // 16), np.int16)
        ar = np.arange(n)
        out[ar % 16, ar // 16] = vals
        return out

    shards = []
    for c in range(ncores):
        fid, zid = cores[c]
        allids = np.concatenate([fid, zid])
        s_all = src_all[allids]
        w0c = max(0, min(int(s_all.min()), n_nodes - srcwin))
        assert int(s_all.max()) - w0c < srcwin, "src window overflow"

        def classprep(ids, nblk, blk, gblks):
            npad = nblk * blk
            s = src_all[ids]
            d = np.asarray(dst_all[ids])
            dord = np.argsort(d, kind='stable')
            ids, s, d = ids[dord], s[dord], d[dord]
            sp = np.zeros(npad, np.int64)
            dp = np.zeros(npad, np.int64)
            perm = np.full(npad, -1, np.int64)
            sp[:len(ids)] = s - w0c
            dp[:len(ids)] = d
            perm[:len(ids)] = ids
            if len(ids) < npad:
                sp[len(ids):] = s[-1] - w0c if len(s) else 0
                # empty class: put pad dsts at each gather's window center
                dp[len(ids):] = d[-1] if len(d) else 0
                if not len(d):
                    ar = np.arange(npad)
                    g0 = (ar // (gblks * blk)) * (gblks * blk)
                    wg = np.minimum(gblks * blk, npad - g0)
                    dp[:] = np.minimum((g0 + wg // 2) * n_nodes // npad,
                                       n_nodes - 1)
            return sp, dp, perm

        fsp, fdp, fperm = classprep(fid, nbf, BLKF, 1)
        zsp, zdp, zperm = classprep(zid, nbz, BLKZ, 1)

        eidx = np.zeros((16, ecols), np.int16)
        for (kind, w, col, elo), base in zip(sched, bases):
            if kind == "f":
                sp = fsp[elo: elo + w]
                dp = fdp[elo: elo + w]
            else:
                sp = zsp[elo: elo + w]
                dp = zdp[elo: elo + w]
            assert dp.min() - base >= 0 and dp.max() - base < DSTWIN, \
                "dst window overflow"
            assert sp.min() >= 0 and sp.max() < srcwin
            eidx[:, col:col + w // 16] = pack_idx(sp)
            eidx[:, col + w // 16:col + w // 8] = pack_idx(dp - base)
        eidx = np.tile(eidx, (8, 1))
        stable = np.ascontiguousarray(table[w0c:w0c + srcwin])
        zstable = np.ascontiguousarray(ztab[w0c:w0c + srcwin])
        shards.append((eidx, stable, zstable, fperm, zperm))

    shared = dict(table=table, ztab=ztab, wpack=wpack)
    meta = dict(nbf=nbf, nbz=nbz, srcwin=srcwin, sched=sched,
                bases=bases, sb2v=sb2v, tbias=tbias, E=E)
    return shared, shards, meta


_BUILD_CACHE = {}


def kernel(z, chemistry, edge, smiles_mask,
           sw1, sb1, sw2, sb2, cw1, cb1, cw2, cb2, mw1, mb1, mw2, mb2,
           path_weights):
    global LAST_EXEC_NS
    from concourse import bass_utils
    from concourse.bass_utils import run_bass_kernel_spmd

    trace = os.environ.get("KERNEL_TRACE", "0") == "1"
    if trace:
        bass_utils.upload_artifacts = lambda tmpdir: tmpdir

    shared, shards, meta = _host_prep(
        z, chemistry, edge, smiles_mask, sw1, sb1, sw2, sb2,
        cw1, cb1, cw2, cb2, mw1, mb1, mw2, mb2, path_weights)
    nbf, nbz, srcwin = meta["nbf"], meta["nbz"], meta["srcwin"]

    key = (N_NODES, nbf, nbz, meta["bases"], srcwin)
    if key not in _BUILD_CACHE:
        _BUILD_CACHE[key] = _build(N_NODES, nbf, nbz, meta["bases"], srcwin)
    nc = _BUILD_CACHE[key]
    in_maps = []
    for c in range(NCORES):
        m = dict(shared)
        m["eidx"], m["stable"], m["zstable"] = \
            shards[c][0], shards[c][1], shards[c][2]
        in_maps.append(m)
    tmpdir = os.environ.get("KERNEL_TRACE_DIR") or None
    res = run_bass_kernel_spmd(nc, in_maps, core_ids=list(range(NCORES)),
                               trace=trace, tmpdir=tmpdir)
    if trace:
        LAST_EXEC_NS = res.exec_time_ns
    results = [res.results[c]["out"].astype(np.float32) for c in range(NCORES)]

    E = meta["E"]
    sb2v, tbias = meta["sb2v"], meta["tbias"]
    result = np.zeros(E, np.float32)
    for c in range(NCORES):
        fperm, zperm = shards[c][3], shards[c][4]
        dev = results[c].reshape(-1)
        t_sc = dev[0:nbf * BLKF] + np.float32(tbias)
        s_sc = dev[nbf * BLKF:] + np.float32(sb2v)
        fv = fperm >= 0
        result[fperm[fv]] = t_sc[fv]
        zv = zperm >= 0
        result[zperm[zv]] = s_sc[zv]
    return result
